# revision 1
# baseline (speedup 1.0000x reference)
"""DiceBCE + OHNM loss for Trainium2 (8 NeuronCores, SPMD data-parallel over batch).

Device side (Bass/Tile, one launch, core b handles batch element b):
  reads preds[b] (f32), computes p = sigmoid(x) — the normalization the
  reference applies before BCE and the quantity whose descending order IS the
  descending order of the negative-class BCE loss (loss|t=0 = softplus(p),
  strictly increasing) — and writes p back to HBM as fp16.

Host side (data-dependent glue, mirrors the reference's host-side numpy):
  top-k hard-negative selection (descending p), positive gather, seeded-RNG
  padding, then the loss values softplus(p) / softplus(-p) and the tiny
  dice + mean reductions over the ~336k selected elements.
"""

import numpy as np

B, C, D, H, W = 8, 1, 128, 128, 128
P = 128
FREE = (C * D * H * W) // P  # 16384 elements per partition per core
N_TILES = 4
TILE_W = FREE // N_TILES  # 4096
EPS = 1e-10
OHNM_RATIO = 3
DEFAULT_NEG_PERC = 0.1

_CACHE = {}


def _build_nc():
    """Raw-Bass (no TileContext — saves the kernel-tail drain/barrier ~7us).

    All 4 input tiles + 4 output tiles stay resident in SBUF (96KB/partition),
    so there is no buffer reuse and the semaphore protocol is trivial:
      sync:   issue the 4 input DMAs back-to-back (inputs get HBM priority),
              then issue each output DMA as its sigmoid completes,
              then wait for all output DMAs to land.
      scalar: per tile, wait for its input DMA, run one fp32->fp16 Sigmoid.
    """
    import contextlib

    from concourse import bacc, mybir

    nc = bacc.Bacc("TRN2", target_bir_lowering=False, debug=False, num_devices=B)
    x = nc.dram_tensor("preds", [P, FREE], mybir.dt.float32, kind="ExternalInput").ap()
    po = nc.dram_tensor("p", [P, FREE], mybir.dt.float16, kind="ExternalOutput").ap()

    with contextlib.ExitStack() as ctx:
        xts = [ctx.enter_context(nc.sbuf_tensor(f"xt{i}", [P, TILE_W], mybir.dt.float32))
               for i in range(N_TILES)]
        pts = [ctx.enter_context(nc.sbuf_tensor(f"pt{i}", [P, TILE_W], mybir.dt.float16))
               for i in range(N_TILES)]
        in_sem = ctx.enter_context(nc.semaphore("in_sem"))
        act_sem = ctx.enter_context(nc.semaphore("act_sem"))
        out_sem = ctx.enter_context(nc.semaphore("out_sem"))
        block = ctx.enter_context(nc.Block())

        @block.sync
        def _(sync):
            for i in range(N_TILES):
                sync.dma_start(
                    xts[i][:, :], x[:, i * TILE_W:(i + 1) * TILE_W]
                ).then_inc(in_sem, 16)
            for i in range(N_TILES):
                sync.wait_ge(act_sem, i + 1)
                sync.dma_start(
                    po[:, i * TILE_W:(i + 1) * TILE_W], pts[i][:, :]
                ).then_inc(out_sem, 16)
            sync.wait_ge(out_sem, N_TILES * 16)

        @block.scalar
        def _(scalar):
            for i in range(N_TILES):
                scalar.wait_ge(in_sem, (i + 1) * 16)
                nc.scalar.activation(
                    pts[i][:, :], xts[i][:, :], mybir.ActivationFunctionType.Sigmoid
                ).then_inc(act_sem, 1)
    nc.compile()
    return nc


def _get_nc():
    if "nc" not in _CACHE:
        _CACHE["nc"] = _build_nc()
    return _CACHE["nc"]


def run_device(preds, targs=None, trace=False, nc=None):
    """Run the SPMD bass kernel on cores 0..7; returns (p_full, BassKernelResults)."""
    from concourse.bass_utils import run_bass_kernel_spmd

    if nc is None:
        nc = _get_nc()
    in_maps = []
    for b in range(B):
        in_maps.append({
            "preds": np.ascontiguousarray(preds[b].reshape(P, FREE), dtype=np.float32),
        })
    try:
        res = run_bass_kernel_spmd(nc, in_maps, core_ids=list(range(B)), trace=trace)
    except Exception:
        # transient device faults (e.g. NRT_EXEC_UNIT_UNRECOVERABLE) usually
        # clear after the runtime resets the cores; one retry is cheap
        import time
        time.sleep(30)
        res = run_bass_kernel_spmd(nc, in_maps, core_ids=list(range(B)), trace=trace)
    p = np.stack([res.results[b]["p"] for b in range(B)])
    return p.reshape(B, C, D, H, W), res


def _host_finish(preds, targs, pmap):
    """Mirror of the reference's host-side get_idxs/pad + dice/mean reductions."""
    x = np.asarray(preds).reshape(-1)
    t = np.asarray(targs).reshape(-1)
    pf = np.asarray(pmap).reshape(-1)
    numel = t.size
    n_pos = int(t.sum())
    n_neg = numel - n_pos
    if n_pos == 0:
        n_hns = int(DEFAULT_NEG_PERC * n_neg)
    else:
        n_hns = min(n_pos * OHNM_RATIO, n_neg)

    # rank negatives: descending loss == descending p == descending x
    # (loss|t=0 = softplus(p), p = sigmoid(x), both strictly increasing).
    # Sorting by x equals sorting by the device fp16 p-map with x breaking the
    # quantization ties, and reproduces the reference's f32-loss order exactly
    # up to f32 rounding ties.
    neg_x = x[t == 0]
    if n_hns > 0:
        if n_hns < neg_x.size:
            part = np.argpartition(-neg_x, n_hns - 1)[:n_hns]
        else:
            part = np.arange(neg_x.size)
        hns_idxs = part[np.argsort(-neg_x[part], kind="stable")]
    else:
        hns_idxs = np.empty(0, dtype=np.int64)
    pos_idxs = np.nonzero(t == 1)[0]
    idxs = np.concatenate([hns_idxs, pos_idxs]).astype(np.int64)
    n_needed = len(idxs) % (B * C)
    if n_needed != 0:
        mask = np.ones(numel, dtype=bool)
        mask[idxs] = False
        remaining = np.nonzero(mask)[0]
        w = remaining.astype(np.float64)
        rng = np.random.default_rng(0)
        extra = rng.choice(remaining, size=n_needed, replace=False, p=w / w.sum())
        idxs = np.concatenate([idxs, extra.astype(np.int64)])

    x_sel = x[idxs].astype(np.float64)
    p_sel = 1.0 / (1.0 + np.exp(-x_sel))          # sigmoid(preds) at selected, exact
    t_sel = t[idxs].astype(np.float64)
    # loss at selected sites: t=0 -> softplus(p) from the device map (the map
    # the ranking ran on); t=1 -> softplus(-p) exact from x
    pq_sel = pf[idxs].astype(np.float64)
    loss_sel = np.where(
        t_sel == 0, np.log1p(np.exp(pq_sel)), np.log1p(np.exp(-p_sel))
    )

    p2 = (1.0 / (1.0 + np.exp(-p_sel))).reshape(B * C, -1)   # dice re-sigmoids
    ts = t_sel.reshape(B * C, -1)
    inter = (p2 * ts).sum(axis=1)
    denom = p2.sum(axis=1) + ts.sum(axis=1)
    dice = np.mean(1.0 - (2.0 * inter + EPS) / (denom + EPS))
    return np.float32(dice + loss_sel.mean())


def kernel(preds, targs):
    preds = np.asarray(preds, dtype=np.float32)
    targs = np.asarray(targs, dtype=np.int32)
    assert preds.shape == (B, C, D, H, W) and targs.shape == (B, C, D, H, W)
    pmap, _ = run_device(preds, trace=False)
    return _host_finish(preds, targs, pmap)



# revision 2
# speedup vs baseline: 1.5115x; 1.5115x over previous
"""DiceBCE + OHNM loss for Trainium2 (8 NeuronCores, SPMD data-parallel over batch).

Device side (raw Bass, one launch, core b handles batch element b):
  reads preds[b] staged as fp16 (the 2e-2 tolerance leaves orders of magnitude
  of headroom for mixed precision; |x|<6 so fp16 keeps ~2^-11 relative error),
  computes p = sigmoid(x) on the ACT engine, and writes the p map back to HBM
  as fp8-e3m4 (1 byte/elt; RNE quantization error averages out across the
  ~250k selected hard-negative sites that consume the map).

Host side (data-dependent glue, mirrors the reference's host-side numpy):
  top-k hard-negative selection (descending x == descending loss), positive
  gather, seeded-RNG padding, then the loss values softplus(p) at negative
  sites from the device map / softplus(-p) exact at positives, and the tiny
  dice + mean reductions over the ~336k selected elements.

Perf notes (vs the 41.9us fp32-in/fp16-out baseline):
  - traffic per core drops 12MB -> 6MB; the ACT sigmoid pass (~15us for 2M
    elements at 1 elem/lane/cycle @1.2GHz) becomes the bottleneck.
  - graded tile widths [2048,4096,4096,4096,2048]: small first tile so ACT
    starts right after its table load, small last tile to shorten the final
    writeback tail.
  - ACT issues its own output DMAs (HWDGE on the ACT queue): no cross-engine
    semaphore hop, and writes drain on a separate ring from the input reads.
"""

import numpy as np

B, C, D, H, W = 8, 1, 128, 128, 128
P = 128
FREE = (C * D * H * W) // P  # 16384 elements per partition per core
TILE_WIDTHS = [2048, 4096, 4096, 4096, 2048]
assert sum(TILE_WIDTHS) == FREE
EPS = 1e-10
OHNM_RATIO = 3
DEFAULT_NEG_PERC = 0.1

_CACHE = {}


def _build_nc():
    import contextlib

    from concourse import bacc, mybir

    nc = bacc.Bacc("TRN2", target_bir_lowering=False, debug=False, num_devices=B)
    x = nc.dram_tensor("preds", [P, FREE], mybir.dt.float16, kind="ExternalInput").ap()
    po = nc.dram_tensor("p", [P, FREE], mybir.dt.float8e3, kind="ExternalOutput").ap()

    offs = [sum(TILE_WIDTHS[:i]) for i in range(len(TILE_WIDTHS))]
    n = len(TILE_WIDTHS)

    with contextlib.ExitStack() as ctx:
        xts = [ctx.enter_context(nc.sbuf_tensor(f"xt{i}", [P, w], mybir.dt.float16))
               for i, w in enumerate(TILE_WIDTHS)]
        pts = [ctx.enter_context(nc.sbuf_tensor(f"pt{i}", [P, w], mybir.dt.float8e3))
               for i, w in enumerate(TILE_WIDTHS)]
        in_sem = ctx.enter_context(nc.semaphore("in_sem"))
        out_sem = ctx.enter_context(nc.semaphore("out_sem"))
        block = ctx.enter_context(nc.Block(no_gpsimd_drain=True))

        @block.sync
        def _(sync):
            for i in range(n):
                sync.dma_start(
                    xts[i][:, :], x[:, offs[i]:offs[i] + TILE_WIDTHS[i]]
                ).then_inc(in_sem, 16)

        @block.scalar
        def _(scalar):
            for i in range(n):
                scalar.wait_ge(in_sem, (i + 1) * 16)
                nc.scalar.activation(
                    pts[i][:, :], xts[i][:, :], mybir.ActivationFunctionType.Sigmoid
                )
                scalar.dma_start(
                    po[:, offs[i]:offs[i] + TILE_WIDTHS[i]], pts[i][:, :]
                ).then_inc(out_sem, 16)
            scalar.wait_ge(out_sem, n * 16)
    nc.compile()
    return nc


def _get_nc():
    if "nc" not in _CACHE:
        _CACHE["nc"] = _build_nc()
    return _CACHE["nc"]


def run_device(preds, targs=None, trace=False, nc=None):
    """Run the SPMD bass kernel on cores 0..7; returns (p_full, BassKernelResults)."""
    from concourse.bass_utils import run_bass_kernel_spmd

    if nc is None:
        nc = _get_nc()
    in_maps = []
    for b in range(B):
        in_maps.append({
            "preds": np.ascontiguousarray(
                preds[b].reshape(P, FREE), dtype=np.float16
            ),
        })
    try:
        res = run_bass_kernel_spmd(nc, in_maps, core_ids=list(range(B)), trace=trace)
    except Exception:
        # transient device faults (e.g. NRT_EXEC_UNIT_UNRECOVERABLE) usually
        # clear after the runtime resets the cores; one retry is cheap
        import time
        time.sleep(30)
        res = run_bass_kernel_spmd(nc, in_maps, core_ids=list(range(B)), trace=trace)
    p = np.stack([np.asarray(res.results[b]["p"]) for b in range(B)])
    return p.reshape(B, C, D, H, W), res


def _host_finish(preds, targs, pmap):
    """Mirror of the reference's host-side get_idxs/pad + dice/mean reductions."""
    x = np.asarray(preds).reshape(-1)
    t = np.asarray(targs).reshape(-1)
    pf = np.asarray(pmap).reshape(-1)
    numel = t.size
    n_pos = int(t.sum())
    n_neg = numel - n_pos
    if n_pos == 0:
        n_hns = int(DEFAULT_NEG_PERC * n_neg)
    else:
        n_hns = min(n_pos * OHNM_RATIO, n_neg)

    # rank negatives: descending loss == descending p == descending x
    # (loss|t=0 = softplus(p), p = sigmoid(x), both strictly increasing).
    neg_x = x[t == 0]
    if n_hns > 0:
        if n_hns < neg_x.size:
            part = np.argpartition(-neg_x, n_hns - 1)[:n_hns]
        else:
            part = np.arange(neg_x.size)
        hns_idxs = part[np.argsort(-neg_x[part], kind="stable")]
    else:
        hns_idxs = np.empty(0, dtype=np.int64)
    pos_idxs = np.nonzero(t == 1)[0]
    idxs = np.concatenate([hns_idxs, pos_idxs]).astype(np.int64)
    n_needed = len(idxs) % (B * C)
    if n_needed != 0:
        mask = np.ones(numel, dtype=bool)
        mask[idxs] = False
        remaining = np.nonzero(mask)[0]
        w = remaining.astype(np.float64)
        rng = np.random.default_rng(0)
        extra = rng.choice(remaining, size=n_needed, replace=False, p=w / w.sum())
        idxs = np.concatenate([idxs, extra.astype(np.int64)])

    x_sel = x[idxs].astype(np.float64)
    p_sel = 1.0 / (1.0 + np.exp(-x_sel))          # sigmoid(preds) at selected, exact
    t_sel = t[idxs].astype(np.float64)
    # loss at selected sites: t=0 -> softplus(p) from the device map (the map
    # the ranking ran on); t=1 -> softplus(-p) exact from x
    pq_sel = pf[idxs].astype(np.float64)
    loss_sel = np.where(
        t_sel == 0, np.log1p(np.exp(pq_sel)), np.log1p(np.exp(-p_sel))
    )

    p2 = (1.0 / (1.0 + np.exp(-p_sel))).reshape(B * C, -1)   # dice re-sigmoids
    ts = t_sel.reshape(B * C, -1)
    inter = (p2 * ts).sum(axis=1)
    denom = p2.sum(axis=1) + ts.sum(axis=1)
    dice = np.mean(1.0 - (2.0 * inter + EPS) / (denom + EPS))
    return np.float32(dice + loss_sel.mean())


def kernel(preds, targs):
    preds = np.asarray(preds, dtype=np.float32)
    targs = np.asarray(targs, dtype=np.int32)
    assert preds.shape == (B, C, D, H, W) and targs.shape == (B, C, D, H, W)
    pmap, _ = run_device(preds, trace=False)
    return _host_finish(preds, targs, pmap)


# revision 7
# speedup vs baseline: 1.5520x; 1.0268x over previous
"""DiceBCE + OHNM loss for Trainium2 (8 NeuronCores, SPMD data-parallel over batch).

Device side (raw Bass, one launch, core b handles batch element b):
  reads preds[b] staged as fp8-e3m4 (|x| < 6 << 15.5 so e3m4 holds every
  input with absolute step <= 0.25; the 2e-2 tolerance leaves orders of
  magnitude of headroom for mixed precision), computes p = sigmoid(x) on the
  ACT engine, and writes the p map back to HBM as fp8-e3m4 (RNE quantization
  errors of both stages average out across the ~250k selected hard-negative
  sites that consume the map).

Host side (data-dependent glue, mirrors the reference's host-side numpy):
  top-k hard-negative selection (descending x == descending loss), positive
  gather, seeded-RNG padding, then the loss values softplus(p) at negative
  sites from the device map / softplus(-p) exact at positives, and the tiny
  dice + mean reductions over the ~336k selected elements.

Perf notes (vs the 41.9us fp32-in/fp16-out baseline):
  - traffic per core drops 12MB -> 4MB; the ACT sigmoid pass (~15us for 2M
    elements at 1 elem/lane/cycle @1.2GHz) becomes the bottleneck.
  - graded tile widths [2048,4096,4096,4096,2048]: small first tile so ACT
    starts right after its table load, small last tile to shorten the final
    writeback tail.
  - ACT issues its own output DMAs (HWDGE on the ACT queue): no cross-engine
    semaphore hop, and writes drain on a separate ring from the input reads.
"""

import numpy as np

B, C, D, H, W = 8, 1, 128, 128, 128
P = 128
FREE = (C * D * H * W) // P  # 16384 elements per partition per core
TILE_WIDTHS = [2048, 4096, 4096, 4096, 2048]
assert sum(TILE_WIDTHS) == FREE
EPS = 1e-10
OHNM_RATIO = 3
DEFAULT_NEG_PERC = 0.1

_CACHE = {}


def _build_nc():
    import contextlib

    from concourse import bacc, mybir

    nc = bacc.Bacc("TRN2", target_bir_lowering=False, debug=False, num_devices=B)
    x = nc.dram_tensor("preds", [P, FREE], mybir.dt.float8e3, kind="ExternalInput").ap()
    po = nc.dram_tensor("p", [P, FREE], mybir.dt.float8e3, kind="ExternalOutput").ap()

    offs = [sum(TILE_WIDTHS[:i]) for i in range(len(TILE_WIDTHS))]
    n = len(TILE_WIDTHS)

    with contextlib.ExitStack() as ctx:
        xts = [ctx.enter_context(nc.sbuf_tensor(f"xt{i}", [P, w], mybir.dt.float8e3))
               for i, w in enumerate(TILE_WIDTHS)]
        pts = [ctx.enter_context(nc.sbuf_tensor(f"pt{i}", [P, w], mybir.dt.float8e3))
               for i, w in enumerate(TILE_WIDTHS)]
        in_sem = ctx.enter_context(nc.semaphore("in_sem"))
        out_sem = ctx.enter_context(nc.semaphore("out_sem"))
        block = ctx.enter_context(nc.Block(no_gpsimd_drain=True))

        @block.sync
        def _(sync):
            for i in range(n):
                sync.dma_start(
                    xts[i][:, :], x[:, offs[i]:offs[i] + TILE_WIDTHS[i]]
                ).then_inc(in_sem, 16)

        @block.scalar
        def _(scalar):
            for i in range(n):
                scalar.wait_ge(in_sem, (i + 1) * 16)
                nc.scalar.activation(
                    pts[i][:, :], xts[i][:, :], mybir.ActivationFunctionType.Sigmoid
                )
                scalar.dma_start(
                    po[:, offs[i]:offs[i] + TILE_WIDTHS[i]], pts[i][:, :]
                ).then_inc(out_sem, 16)
            scalar.wait_ge(out_sem, n * 16)
    nc.compile()
    return nc


def _get_nc():
    if "nc" not in _CACHE:
        _CACHE["nc"] = _build_nc()
    return _CACHE["nc"]


def run_device(preds, targs=None, trace=False, nc=None):
    """Run the SPMD bass kernel on cores 0..7; returns (p_full, BassKernelResults)."""
    from concourse.bass_utils import run_bass_kernel_spmd

    if nc is None:
        nc = _get_nc()
    import ml_dtypes
    in_maps = []
    for b in range(B):
        in_maps.append({
            "preds": np.ascontiguousarray(preds[b].reshape(P, FREE)).astype(
                ml_dtypes.float8_e3m4
            ),
        })
    try:
        res = run_bass_kernel_spmd(nc, in_maps, core_ids=list(range(B)), trace=trace)
    except Exception:
        # transient device faults (e.g. NRT_EXEC_UNIT_UNRECOVERABLE) usually
        # clear after the runtime resets the cores; one retry is cheap
        import time
        time.sleep(30)
        res = run_bass_kernel_spmd(nc, in_maps, core_ids=list(range(B)), trace=trace)
    p = np.stack([np.asarray(res.results[b]["p"]) for b in range(B)])
    return p.reshape(B, C, D, H, W), res


def _host_finish(preds, targs, pmap):
    """Mirror of the reference's host-side get_idxs/pad + dice/mean reductions."""
    x = np.asarray(preds).reshape(-1)
    t = np.asarray(targs).reshape(-1)
    pf = np.asarray(pmap).reshape(-1)
    numel = t.size
    n_pos = int(t.sum())
    n_neg = numel - n_pos
    if n_pos == 0:
        n_hns = int(DEFAULT_NEG_PERC * n_neg)
    else:
        n_hns = min(n_pos * OHNM_RATIO, n_neg)

    # rank negatives: descending loss == descending p == descending x
    # (loss|t=0 = softplus(p), p = sigmoid(x), both strictly increasing).
    neg_x = x[t == 0]
    if n_hns > 0:
        if n_hns < neg_x.size:
            part = np.argpartition(-neg_x, n_hns - 1)[:n_hns]
        else:
            part = np.arange(neg_x.size)
        hns_idxs = part[np.argsort(-neg_x[part], kind="stable")]
    else:
        hns_idxs = np.empty(0, dtype=np.int64)
    pos_idxs = np.nonzero(t == 1)[0]
    idxs = np.concatenate([hns_idxs, pos_idxs]).astype(np.int64)
    n_needed = len(idxs) % (B * C)
    if n_needed != 0:
        mask = np.ones(numel, dtype=bool)
        mask[idxs] = False
        remaining = np.nonzero(mask)[0]
        w = remaining.astype(np.float64)
        rng = np.random.default_rng(0)
        extra = rng.choice(remaining, size=n_needed, replace=False, p=w / w.sum())
        idxs = np.concatenate([idxs, extra.astype(np.int64)])

    x_sel = x[idxs].astype(np.float64)
    p_sel = 1.0 / (1.0 + np.exp(-x_sel))          # sigmoid(preds) at selected, exact
    t_sel = t[idxs].astype(np.float64)
    # loss at selected sites: t=0 -> softplus(p) from the device map (the map
    # the ranking ran on); t=1 -> softplus(-p) exact from x
    pq_sel = pf[idxs].astype(np.float64)
    loss_sel = np.where(
        t_sel == 0, np.log1p(np.exp(pq_sel)), np.log1p(np.exp(-p_sel))
    )

    p2 = (1.0 / (1.0 + np.exp(-p_sel))).reshape(B * C, -1)   # dice re-sigmoids
    ts = t_sel.reshape(B * C, -1)
    inter = (p2 * ts).sum(axis=1)
    denom = p2.sum(axis=1) + ts.sum(axis=1)
    dice = np.mean(1.0 - (2.0 * inter + EPS) / (denom + EPS))
    return np.float32(dice + loss_sel.mean())


def kernel(preds, targs):
    preds = np.asarray(preds, dtype=np.float32)
    targs = np.asarray(targs, dtype=np.int32)
    assert preds.shape == (B, C, D, H, W) and targs.shape == (B, C, D, H, W)
    pmap, _ = run_device(preds, trace=False)
    return _host_finish(preds, targs, pmap)


# revision 8
# speedup vs baseline: 1.9029x; 1.2261x over previous
"""DiceBCE + OHNM loss for Trainium2 (8 NeuronCores, SPMD data-parallel over batch).

Device side (raw Bass, one launch, core b handles batch element b):
  reads preds[b] staged as fp16 (|x| < 6, so fp16 keeps ~2^-11 relative error;
  the 2e-2 tolerance leaves orders of magnitude of mixed-precision headroom),
  computes p = sigmoid(x) on the ACT engine, and writes the p map back to HBM
  as fp8-e3m4 (1 byte/elt; RNE quantization error averages out across the
  ~250k selected hard-negative sites that consume the map).

Host side (data-dependent glue, mirrors the reference's host-side numpy):
  top-k hard-negative selection (descending x == descending loss), positive
  gather, seeded-RNG padding, then the loss values softplus(p) at negative
  sites from the device map / softplus(-p) exact at positives, and the tiny
  dice + mean reductions over the ~336k selected elements.

Perf notes (vs the 41.9us fp32-in/fp16-out baseline):
  - traffic per core drops 12MB -> 6MB; the ACT sigmoid pass (~14.5us for 2M
    elements at 1 elem/lane/cycle @1.2GHz) is the bottleneck engine.
  - 3 tiles [10240, 4096, 2048]: few ACTIVATEs (each pays a ~352-cycle
    pipeline fill), inputs prefetch during the long first tile so the act
    chain never starves, and the small last tile keeps the writeback tail
    short. ACT issues its own output DMAs (HWDGE, same-engine program order
    guarantees act-before-dma) so writes overlap later activations.
  - the activation bias constant is DMA'd from HBM instead of the framework's
    four const-tile MEMSETs (deleted from the BIR pre-compile): the memsets
    are dead weight for this kernel and would otherwise serialize in front
    of the DMA ramp on the gpsimd engine.
"""

import numpy as np

B, C, D, H, W = 8, 1, 128, 128, 128
P = 128
FREE = (C * D * H * W) // P  # 16384 elements per partition per core
TILE_WIDTHS = [10240, 4096, 2048]
assert sum(TILE_WIDTHS) == FREE
EPS = 1e-10
OHNM_RATIO = 3
DEFAULT_NEG_PERC = 0.1

_CACHE = {}


def _build_nc():
    import contextlib

    from concourse import bacc, mybir

    nc = bacc.Bacc("TRN2", target_bir_lowering=False, debug=False, num_devices=B)
    x = nc.dram_tensor("preds", [P, FREE], mybir.dt.float16, kind="ExternalInput").ap()
    z = nc.dram_tensor("bias0", [P, 1], mybir.dt.float32, kind="ExternalInput").ap()
    po = nc.dram_tensor("p", [P, FREE], mybir.dt.float8e3, kind="ExternalOutput").ap()

    offs = [sum(TILE_WIDTHS[:i]) for i in range(len(TILE_WIDTHS))]
    n = len(TILE_WIDTHS)

    with contextlib.ExitStack() as ctx:
        xts = [ctx.enter_context(nc.sbuf_tensor(f"xt{i}", [P, w], mybir.dt.float16))
               for i, w in enumerate(TILE_WIDTHS)]
        pts = [ctx.enter_context(nc.sbuf_tensor(f"pt{i}", [P, w], mybir.dt.float8e3))
               for i, w in enumerate(TILE_WIDTHS)]
        bt = ctx.enter_context(nc.sbuf_tensor("bt", [P, 1], mybir.dt.float32))
        in_sem = ctx.enter_context(nc.semaphore("in_sem"))
        out_sem = ctx.enter_context(nc.semaphore("out_sem"))
        block = ctx.enter_context(nc.Block(no_gpsimd_drain=True))

        @block.sync
        def _(sync):
            sync.dma_start(bt[:, :], z[:, :]).then_inc(in_sem, 16)
            for i in range(n):
                sync.dma_start(
                    xts[i][:, :], x[:, offs[i]:offs[i] + TILE_WIDTHS[i]]
                ).then_inc(in_sem, 16)

        @block.scalar
        def _(scalar):
            for i in range(n):
                scalar.wait_ge(in_sem, (i + 2) * 16)
                nc.scalar.activation(
                    pts[i][:, :], xts[i][:, :],
                    mybir.ActivationFunctionType.Sigmoid, bias=bt[:, :],
                )
                scalar.dma_start(
                    po[:, offs[i]:offs[i] + TILE_WIDTHS[i]], pts[i][:, :]
                ).then_inc(out_sem, 16)
            scalar.wait_ge(out_sem, n * 16)

    # Drop the framework's const-tile MEMSETs (the activation bias now comes
    # from the bias0 DMA; nothing else reads the const tiles in this kernel).
    for f in nc.m.functions:
        for blk in f.blocks:
            for inst in [i for i in blk.instructions
                         if type(i).__name__ == 'InstMemset']:
                blk.instructions.remove(inst)

    nc.compile()
    return nc


def _get_nc():
    if "nc" not in _CACHE:
        _CACHE["nc"] = _build_nc()
    return _CACHE["nc"]


def run_device(preds, targs=None, trace=False, nc=None):
    """Run the SPMD bass kernel on cores 0..7; returns (p_full, BassKernelResults)."""
    from concourse.bass_utils import run_bass_kernel_spmd

    if nc is None:
        nc = _get_nc()
    zeros = np.zeros((P, 1), dtype=np.float32)
    in_maps = []
    for b in range(B):
        in_maps.append({
            "preds": np.ascontiguousarray(
                preds[b].reshape(P, FREE), dtype=np.float16
            ),
            "bias0": zeros,
        })
    try:
        res = run_bass_kernel_spmd(nc, in_maps, core_ids=list(range(B)), trace=trace)
    except Exception:
        # transient device faults (e.g. NRT_EXEC_UNIT_UNRECOVERABLE) usually
        # clear after the runtime resets the cores; one retry is cheap
        import time
        time.sleep(30)
        res = run_bass_kernel_spmd(nc, in_maps, core_ids=list(range(B)), trace=trace)
    p = np.stack([np.asarray(res.results[b]["p"]) for b in range(B)])
    return p.reshape(B, C, D, H, W), res


def _host_finish(preds, targs, pmap):
    """Mirror of the reference's host-side get_idxs/pad + dice/mean reductions."""
    x = np.asarray(preds).reshape(-1)
    t = np.asarray(targs).reshape(-1)
    pf = np.asarray(pmap).reshape(-1)
    numel = t.size
    n_pos = int(t.sum())
    n_neg = numel - n_pos
    if n_pos == 0:
        n_hns = int(DEFAULT_NEG_PERC * n_neg)
    else:
        n_hns = min(n_pos * OHNM_RATIO, n_neg)

    # rank negatives: descending loss == descending p == descending x
    # (loss|t=0 = softplus(p), p = sigmoid(x), both strictly increasing).
    neg_x = x[t == 0]
    if n_hns > 0:
        if n_hns < neg_x.size:
            part = np.argpartition(-neg_x, n_hns - 1)[:n_hns]
        else:
            part = np.arange(neg_x.size)
        hns_idxs = part[np.argsort(-neg_x[part], kind="stable")]
    else:
        hns_idxs = np.empty(0, dtype=np.int64)
    pos_idxs = np.nonzero(t == 1)[0]
    idxs = np.concatenate([hns_idxs, pos_idxs]).astype(np.int64)
    n_needed = len(idxs) % (B * C)
    if n_needed != 0:
        mask = np.ones(numel, dtype=bool)
        mask[idxs] = False
        remaining = np.nonzero(mask)[0]
        w = remaining.astype(np.float64)
        rng = np.random.default_rng(0)
        extra = rng.choice(remaining, size=n_needed, replace=False, p=w / w.sum())
        idxs = np.concatenate([idxs, extra.astype(np.int64)])

    x_sel = x[idxs].astype(np.float64)
    p_sel = 1.0 / (1.0 + np.exp(-x_sel))          # sigmoid(preds) at selected, exact
    t_sel = t[idxs].astype(np.float64)
    # loss at selected sites: t=0 -> softplus(p) from the device map (the map
    # the ranking ran on); t=1 -> softplus(-p) exact from x
    pq_sel = pf[idxs].astype(np.float64)
    loss_sel = np.where(
        t_sel == 0, np.log1p(np.exp(pq_sel)), np.log1p(np.exp(-p_sel))
    )

    p2 = (1.0 / (1.0 + np.exp(-p_sel))).reshape(B * C, -1)   # dice re-sigmoids
    ts = t_sel.reshape(B * C, -1)
    inter = (p2 * ts).sum(axis=1)
    denom = p2.sum(axis=1) + ts.sum(axis=1)
    dice = np.mean(1.0 - (2.0 * inter + EPS) / (denom + EPS))
    return np.float32(dice + loss_sel.mean())


def kernel(preds, targs):
    preds = np.asarray(preds, dtype=np.float32)
    targs = np.asarray(targs, dtype=np.int32)
    assert preds.shape == (B, C, D, H, W) and targs.shape == (B, C, D, H, W)
    pmap, _ = run_device(preds, trace=False)
    return _host_finish(preds, targs, pmap)
